# revision 6
# baseline (speedup 1.0000x reference)
"""DenseCLIP contrastive-loss kernel for one TRN2 chip (8 NeuronCores).

Strategy: data-parallel over the video (y) axis of the score tensor.
Each core holds the full text latents [64,32,512] and its own shard of 8
videos [8,197,512]; it computes the [2048, 8*197] late-interaction score
matrix on the tensor engine, the max over image tokens on the vector
engine (straight out of PSUM), and the masked mean over text tokens as a
small accumulating matmul against a host-built mask-weight matrix.  The
per-core output is the [64, 8] text_to_image slab; the host concatenates
the 8 slabs and finishes the (tiny) softmax-style loss.

Host-side work is layout only (transpose to channel-major, bf16 cast,
zero-padding of the image-token axis, mask -> weight matrix); all
floating-point work of the module itself (normalization, scores, max,
masked mean) runs on the NeuronCores.
"""

import sys

sys.path.insert(0, "/opt/trn_rl_repo")

import numpy as np
import ml_dtypes

TEMPERATURE = 0.07
LOG_EPS = 1e-20
MEAN_EPS = 1e-6

B = 64          # text batch == video batch
T1 = 33         # 1 + text seq len
I1 = 197        # 1 + image tokens
C = 512         # embed dim
NCORES = 8
T = T1 - 1      # 32 latent tokens
YS = B // NCORES  # 8 videos per core
IPAD = 200      # image tokens padded for alignment
M = B * T       # 2048 score rows per core
KC = C // 128   # 4 contraction chunks
MT = M // 128   # 16 row tiles

_CACHE: dict = {}


def _split_multi_waits(nc):
    """walrus in this container rejects >1 semaphore wait per instruction
    (setupSyncWait: 'Too many sync wait commands').  Hoist extra waits onto
    NoOp instructions inserted just before the offender on the same engine —
    engine streams execute in order, so the barrier semantics are identical."""
    import copy

    from concourse import mybir

    # Template NoOps per engine (emitted then detached from their block).
    builders = {
        mybir.EngineType.PE: nc.tensor,
        mybir.EngineType.Activation: nc.scalar,
        mybir.EngineType.DVE: nc.vector,
        mybir.EngineType.SP: nc.sync,
        mybir.EngineType.Pool: nc.gpsimd,
    }
    templates = {}
    for eng, b in builders.items():
        inst = b.nop(hint="waitsplit").ins
        for bb in nc.m.functions[0].blocks:
            if inst in bb.instructions:
                lst = list(bb.instructions)
                lst.remove(inst)
                bb.instructions = lst
        templates[eng] = inst

    n_id = [0]
    for bb in nc.m.functions[0].blocks:
        new_list = []
        changed = False
        for inst in bb.instructions:
            si = inst.sync_info
            waits = list(si.on_wait) if si and si.on_wait else []
            if len(waits) > 1 and inst.engine in templates:
                changed = True
                for w in waits[:-1]:
                    nop = copy.copy(templates[inst.engine])
                    nop.name = f"I-waitsplit-{n_id[0]}"
                    n_id[0] += 1
                    nop.sync_info = mybir.SyncInfo(on_wait=[w], on_update=[])
                    nc.register_instruction(nop, overwrite=True)
                    new_list.append(nop)
                inst.sync_info = mybir.SyncInfo(
                    on_wait=[waits[-1]], on_update=list(si.on_update or [])
                )
            new_list.append(inst)
        if changed:
            bb.instructions = new_list


def build_nc():
    """Build the single-core Bass program (same program runs SPMD on 8 cores)."""
    import concourse.bass as bass
    import concourse.tile as tile
    from concourse import mybir

    f32 = mybir.dt.float32
    bf16 = mybir.dt.bfloat16
    X = mybir.AxisListType.X

    nc = bass.Bass("TRN2", target_bir_lowering=False, debug=False, num_devices=1)

    tt_lat = nc.dram_tensor("tt_lat", [C, B, T], bf16, kind="ExternalInput").ap()
    tt_cls = nc.dram_tensor("tt_cls", [C, B], bf16, kind="ExternalInput").ap()
    vt = nc.dram_tensor("vt", [C, YS, IPAD], bf16, kind="ExternalInput").ap()
    wsel = nc.dram_tensor("wsel", [M, B], f32, kind="ExternalInput").ap()
    out = nc.dram_tensor("out", [B, YS], f32, kind="ExternalOutput").ap()

    with tile.TileContext(nc) as tc:
        with (
            tc.tile_pool(name="ins", bufs=1) as ins_pool,
            tc.tile_pool(name="ops", bufs=1) as ops_pool,
            tc.tile_pool(name="norm", bufs=1) as norm_pool,
            tc.tile_pool(name="sq", bufs=2) as sq_pool,
            tc.tile_pool(name="t2i", bufs=4) as t2i_pool,
            tc.tile_pool(name="osb", bufs=1) as osb_pool,
            tc.tile_pool(name="simps", bufs=3, space="PSUM") as simps_pool,
            tc.tile_pool(name="lossps", bufs=1, space="PSUM") as lossps_pool,
        ):
            # ---- load inputs (channel-major: c on partitions) ----
            ttl = ins_pool.tile([128, KC, B, T], bf16, tag="ttl")
            nc.sync.dma_start(
                out=ttl[:], in_=tt_lat.rearrange("(k p) b t -> p k b t", p=128)
            )
            cls = ins_pool.tile([128, KC, B], bf16, tag="cls")
            nc.sync.dma_start(
                out=cls[:], in_=tt_cls.rearrange("(k p) b -> p k b", p=128)
            )
            vtt = ins_pool.tile([128, KC, YS, IPAD], bf16, tag="vtt")
            nc.sync.dma_start(
                out=vtt[:], in_=vt.rearrange("(k p) y i -> p k y i", p=128)
            )
            wt = ins_pool.tile([128, MT, B], f32, tag="wt")
            nc.sync.dma_start(
                out=wt[:], in_=wsel.rearrange("(m p) x -> p m x", p=128)
            )

            # ---- normalization factors ----
            # text: rnt[c, b] = 1/sqrt(cls^2 + sum_t lat^2)
            # video: rnv[c, y] = TEMPERATURE/sqrt(sum_i v^2)   (pads are 0)
            ss_t = norm_pool.tile([128, KC, B], f32, tag="ss_t")
            ss_v = norm_pool.tile([128, KC, YS], f32, tag="ss_v")
            rnt = norm_pool.tile([128, KC, B], f32, tag="rnt")
            rnv = norm_pool.tile([128, KC, YS], f32, tag="rnv")
            clssq = norm_pool.tile([128, KC, B], f32, tag="clssq")
            rnt_x = ops_pool.tile([128, KC, B, T], bf16, tag="rnt_x")
            rnv_x = ops_pool.tile([128, KC, YS, IPAD], bf16, tag="rnv_x")
            tl = ops_pool.tile([128, KC, B, T], bf16, tag="tl")
            ve = ops_pool.tile([128, KC, YS, IPAD], bf16, tag="ve")

            SQ = mybir.ActivationFunctionType.Square
            SQRT = mybir.ActivationFunctionType.Sqrt
            CP = mybir.ActivationFunctionType.Copy

            for k in range(KC):
                # text
                sq = sq_pool.tile([128, B * T], f32, tag="sq")
                nc.scalar.activation(
                    sq[:], ttl[:, k].rearrange("p b t -> p (b t)"), SQ
                )
                nc.vector.reduce_sum(
                    out=ss_t[:, k], in_=sq.rearrange("p (b t) -> p b t", b=B), axis=X
                )
                nc.vector.tensor_mul(clssq[:, k], cls[:, k], cls[:, k])
                nc.vector.tensor_add(ss_t[:, k], ss_t[:, k], clssq[:, k])
                nc.scalar.activation(ss_t[:, k], ss_t[:, k], SQRT)
                nc.vector.reciprocal(rnt[:, k], ss_t[:, k])
                nc.scalar.activation(
                    rnt_x[:, k],
                    rnt[:, k].unsqueeze(2).broadcast_to((128, B, T)),
                    CP,
                )
                nc.vector.tensor_mul(tl[:, k], ttl[:, k], rnt_x[:, k])

                # video (temperature folded in: 1/sqrt(x/temp^2) = temp/sqrt(x))
                sqv = sq_pool.tile([128, B * T], f32, tag="sq")
                sqv_v = sqv[:, : YS * IPAD]
                nc.scalar.activation(
                    sqv_v, vtt[:, k].rearrange("p y i -> p (y i)"), SQ
                )
                nc.vector.reduce_sum(
                    out=ss_v[:, k],
                    in_=sqv_v.rearrange("p (y i) -> p y i", y=YS),
                    axis=X,
                )
                nc.scalar.activation(
                    ss_v[:, k], ss_v[:, k], SQRT, scale=1.0 / (TEMPERATURE**2)
                )
                nc.vector.reciprocal(rnv[:, k], ss_v[:, k])
                nc.scalar.activation(
                    rnv_x[:, k],
                    rnv[:, k].unsqueeze(2).broadcast_to((128, YS, IPAD)),
                    CP,
                )
                nc.vector.tensor_mul(ve[:, k], vtt[:, k], rnv_x[:, k])

            # ---- scores + max over image tokens + masked mean ----
            tlf = tl.rearrange("p k b t -> p k (b t)")
            loss_ps = lossps_pool.tile([B, YS], f32, tag="loss")

            for m in range(MT):
                ps = [
                    simps_pool.tile(
                        [128, 2, 512], f32, tag="ps", name=f"ps{m}_{h}"
                    )
                    for h in range(2)
                ]
                for k in range(KC):
                    lhsT = tlf[:, k, m * 128 : (m + 1) * 128]
                    for j in range(4):  # 2 videos per psum bank
                        nc.tensor.matmul(
                            ps[j // 2][:, j % 2, : 2 * IPAD],
                            lhsT,
                            ve[:, k, 2 * j : 2 * j + 2].rearrange(
                                "p y i -> p (y i)"
                            ),
                            start=(k == 0),
                            stop=(k == KC - 1),
                            skip_group_check=True,
                        )
                t2i_m = t2i_pool.tile([128, YS], f32, tag="t2i")
                for h in range(2):
                    nc.vector.reduce_max(
                        out=t2i_m[:, 4 * h : 4 * h + 4].rearrange(
                            "p (a y) -> p a y", a=2
                        ),
                        in_=ps[h][:, :, : 2 * IPAD]
                        .rearrange("p a (y i) -> p a y i", y=2)[:, :, :, :I1],
                        axis=X,
                    )
                nc.tensor.matmul(
                    loss_ps[:, :],
                    wt[:, m],
                    t2i_m[:],
                    start=(m == 0),
                    stop=(m == MT - 1),
                    skip_group_check=True,
                )

            osb = osb_pool.tile([B, YS], f32, tag="osb")
            nc.scalar.activation(osb[:], loss_ps[:], CP)
            nc.sync.dma_start(out=out, in_=osb[:])

    _split_multi_waits(nc)
    return nc


def _get_nc():
    if "nc" not in _CACHE:
        _CACHE["nc"] = build_nc()
    return _CACHE["nc"]


def host_prep(text_embeds, video_embeds, text_attn_mask):
    """Layout-only host prep: transpose to channel-major, bf16, pad, W."""
    bf16 = ml_dtypes.bfloat16
    tt = np.ascontiguousarray(text_embeds.transpose(2, 0, 1))  # [C, B, T1]
    tt_lat = np.ascontiguousarray(tt[:, :, 1:]).astype(bf16)
    tt_cls = np.ascontiguousarray(tt[:, :, 0]).astype(bf16)

    vt = video_embeds.transpose(2, 0, 1)  # [C, B, I1]
    vt_pad = np.zeros((C, B, IPAD), np.float32)
    vt_pad[:, :, :I1] = vt
    vt_pad = vt_pad.astype(bf16)

    mask = text_attn_mask[:, 1:].astype(np.float32)  # [B, T]
    cnt = np.maximum(mask.sum(axis=1), MEAN_EPS).astype(np.float32)
    wsel = np.zeros((M, B), np.float32)
    for x in range(B):
        wsel[x * T : (x + 1) * T, x] = mask[x] / cnt[x]

    in_maps = []
    for i in range(NCORES):
        in_maps.append(
            {
                "tt_lat": tt_lat,
                "tt_cls": tt_cls,
                "vt": np.ascontiguousarray(vt_pad[:, i * YS : (i + 1) * YS, :]),
                "wsel": wsel,
            }
        )
    return in_maps


def host_finish(t2i_slabs):
    """exp / diag / sum / log / mean on the [64, 64] text_to_image matrix."""
    t2i = np.concatenate(t2i_slabs, axis=1).astype(np.float32)  # [B, B]
    e = np.exp(t2i)
    pos = np.diagonal(e)
    den = e.sum(axis=-1)
    loss = -np.log(pos / den + LOG_EPS).mean()
    return np.array([loss], dtype=np.float32)


def kernel(text_embeds, video_embeds, text_attn_mask):
    from concourse import bass_utils

    nc = _get_nc()
    in_maps = host_prep(
        np.asarray(text_embeds, np.float32),
        np.asarray(video_embeds, np.float32),
        np.asarray(text_attn_mask),
    )
    res = bass_utils.run_bass_kernel_spmd(
        nc, in_maps, core_ids=list(range(NCORES))
    )
    return host_finish([res.results[i]["out"] for i in range(NCORES)])


# revision 10
# speedup vs baseline: 1.0229x; 1.0229x over previous
"""DenseCLIP contrastive-loss kernel for one TRN2 chip (8 NeuronCores).

Strategy: data-parallel over the video (y) axis of the score tensor.
Each core holds the full text latents and its own shard of 8 videos; it
computes the [2048, 8*197] late-interaction score matrix on the tensor
engine, the max over image tokens on the vector engine (straight out of
PSUM), and the masked mean over text tokens as a small accumulating
matmul against a host-built mask-weight matrix.  The per-core output is
the [64, 8] text_to_image slab; the host concatenates the 8 slabs and
finishes the (tiny) softmax-style loss.

The sum-of-squares norms are computed on the tensor engine as selector
matmuls over natural-layout (token-major) copies of the inputs — this
keeps the PE warm through the normalization phase and keeps the vector
engine free for the max-reduction, which only it can do.

Host-side work is layout only (transposes, bf16 cast, zero padding,
mask -> weight matrix, 0/1 selector matrices); all floating-point work
of the module itself (normalization, scores, max, masked mean) runs on
the NeuronCores.
"""

import sys

sys.path.insert(0, "/opt/trn_rl_repo")

import numpy as np
import ml_dtypes

TEMPERATURE = 0.07
LOG_EPS = 1e-20
MEAN_EPS = 1e-6

B = 64          # text batch == video batch
T1 = 33         # 1 + text seq len
I1 = 197        # 1 + image tokens
C = 512         # embed dim
NCORES = 8
T = T1 - 1      # 32 latent tokens
YS = B // NCORES  # 8 videos per core
IPAD = 200      # image tokens padded for alignment
M = B * T       # 2048 score rows per core
KC = C // 128   # 4 contraction chunks
MT = M // 128   # 16 row tiles

TNR = B * T1            # 2112 natural text rows (incl CLS)
TNT = (TNR + 127) // 128  # 17 natural text row tiles
VNR = YS * I1           # 1576 natural video rows
VNT = (VNR + 127) // 128  # 13 natural video row tiles

_CACHE: dict = {}


def _split_multi_waits(nc):
    """walrus in this container rejects >1 semaphore wait per instruction
    (setupSyncWait: 'Too many sync wait commands').  Hoist extra waits onto
    NoOp instructions inserted just before the offender on the same engine —
    engine streams execute in order, so the barrier semantics are identical."""
    import copy

    from concourse import mybir

    builders = {
        mybir.EngineType.PE: nc.tensor,
        mybir.EngineType.Activation: nc.scalar,
        mybir.EngineType.DVE: nc.vector,
        mybir.EngineType.SP: nc.sync,
        mybir.EngineType.Pool: nc.gpsimd,
    }
    templates = {}
    for eng, b in builders.items():
        inst = b.nop(hint="waitsplit").ins
        for bb in nc.m.functions[0].blocks:
            if inst in bb.instructions:
                lst = list(bb.instructions)
                lst.remove(inst)
                bb.instructions = lst
        templates[eng] = inst

    n_id = [0]
    for bb in nc.m.functions[0].blocks:
        new_list = []
        changed = False
        for inst in bb.instructions:
            si = inst.sync_info
            waits = list(si.on_wait) if si and si.on_wait else []
            if len(waits) > 1 and inst.engine in templates:
                changed = True
                for w in waits[:-1]:
                    nop = copy.copy(templates[inst.engine])
                    nop.name = f"I-waitsplit-{n_id[0]}"
                    n_id[0] += 1
                    nop.sync_info = mybir.SyncInfo(on_wait=[w], on_update=[])
                    nc.register_instruction(nop, overwrite=True)
                    new_list.append(nop)
                inst.sync_info = mybir.SyncInfo(
                    on_wait=[waits[-1]], on_update=list(si.on_update or [])
                )
            new_list.append(inst)
        if changed:
            bb.instructions = new_list


def build_nc():
    """Build the single-core Bass program (same program runs SPMD on 8 cores)."""
    import concourse.bass as bass
    import concourse.tile as tile
    from concourse import mybir

    f32 = mybir.dt.float32
    bf16 = mybir.dt.bfloat16
    X = mybir.AxisListType.X
    SQ = mybir.ActivationFunctionType.Square
    SQRT = mybir.ActivationFunctionType.Sqrt
    CP = mybir.ActivationFunctionType.Copy

    nc = bass.Bass("TRN2", target_bir_lowering=False, debug=False, num_devices=1)

    tt_lat = nc.dram_tensor("tt_lat", [C, B, T], bf16, kind="ExternalInput").ap()
    vt = nc.dram_tensor("vt", [C, YS, IPAD], bf16, kind="ExternalInput").ap()
    tnat = nc.dram_tensor("tnat", [TNT * 128, C], bf16, kind="ExternalInput").ap()
    vnat = nc.dram_tensor("vnat", [VNT * 128, C], bf16, kind="ExternalInput").ap()
    sel_t = nc.dram_tensor("sel_t", [TNT * 128, B], bf16, kind="ExternalInput").ap()
    sel_v = nc.dram_tensor("sel_v", [VNT * 128, YS], bf16, kind="ExternalInput").ap()
    wsel = nc.dram_tensor("wsel", [M, B], f32, kind="ExternalInput").ap()
    out = nc.dram_tensor("out", [B, YS], f32, kind="ExternalOutput").ap()

    with tile.TileContext(nc) as tc:
        with (
            tc.tile_pool(name="lossps", bufs=1, space="PSUM") as lossps_pool,
            tc.tile_pool(name="wup", bufs=1, space="PSUM") as wup_pool,
            tc.tile_pool(name="ins", bufs=1) as ins_pool,
            tc.tile_pool(name="nat", bufs=1) as nat_pool,
            tc.tile_pool(name="ops", bufs=1) as ops_pool,
            tc.tile_pool(name="norm", bufs=1) as norm_pool,
            tc.tile_pool(name="t2i", bufs=4) as t2i_pool,
            tc.tile_pool(name="osb", bufs=1) as osb_pool,
        ):
            loss_ps = lossps_pool.tile([B, YS], f32, tag="loss")
            wup_ps = wup_pool.tile([128, 512], f32, tag="wup")

            # ---- selector matrices + natural-layout tokens (norm inputs) ----
            slt = ins_pool.tile([128, TNT, B], bf16, tag="slt")
            nc.sync.dma_start(
                out=slt[:], in_=sel_t.rearrange("(j p) b -> p j b", p=128)
            )
            slv = ins_pool.tile([128, VNT, YS], bf16, tag="slv")
            nc.sync.dma_start(
                out=slv[:], in_=sel_v.rearrange("(j p) y -> p j y", p=128)
            )

            tn = nat_pool.tile([128, TNT, C], bf16, tag="tn")
            tnr = tnat.rearrange("(j p) c -> p j c", p=128)
            vn = nat_pool.tile([128, VNT, C], bf16, tag="vn")
            vnr = vnat.rearrange("(j p) c -> p j c", p=128)
            # natural-row groups: (kind, j0, j1)
            groups = []
            for g in range(5):
                j0, j1 = 4 * g, min(4 * g + 4, TNT)
                if j0 < j1:
                    groups.append(("t", j0, j1))
            for g in range(4):
                j0, j1 = 4 * g, min(4 * g + 4, VNT)
                if j0 < j1:
                    groups.append(("v", j0, j1))
            groups.sort(key=lambda g: g[1])  # interleave text/video
            for kind, j0, j1 in groups:
                src = tnr if kind == "t" else vnr
                dst = tn if kind == "t" else vn
                nc.sync.dma_start(out=dst[:, j0:j1], in_=src[:, j0:j1])

            # ---- matmul operands (channel-major) ----
            ttl = ops_pool.tile([128, KC, B, T], bf16, tag="ttl")
            ttlr = tt_lat.rearrange("(k p) b t -> p k b t", p=128)
            vtt = ops_pool.tile([128, KC, YS, IPAD], bf16, tag="vtt")
            vttr = vt.rearrange("(k p) y i -> p k y i", p=128)
            for k in range(KC):
                nc.sync.dma_start(out=vtt[:, k], in_=vttr[:, k])
                nc.sync.dma_start(out=ttl[:, k], in_=ttlr[:, k])
            wt = ins_pool.tile([128, MT, B], f32, tag="wt")
            nc.sync.dma_start(
                out=wt[:], in_=wsel.rearrange("(m p) x -> p m x", p=128)
            )

            # ---- sum-of-squares via selector matmuls (ss lands [c, b]) ----
            # per chunk k: one PSUM bank holding [128, 64 text | 8 video]
            with tc.tile_pool(name="ssps", bufs=1, space="PSUM") as ssps_pool:
                ss_ps = ssps_pool.tile([128, KC, B + YS], f32, tag="ssps")
                sqt = nat_pool.tile([128, TNT, C], bf16, tag="sqt")
                sqv = nat_pool.tile([128, VNT, C], bf16, tag="sqv")
                for kind, j0, j1 in groups:
                    if kind == "t":
                        nc.scalar.activation(
                            sqt[:, j0:j1].rearrange("p j c -> p (j c)"),
                            tn[:, j0:j1].rearrange("p j c -> p (j c)"),
                            SQ,
                        )
                        for j in range(j0, j1):
                            for k in range(KC):
                                # start=True clears the whole PSUM bank, so
                                # only the very first matmul into the shared
                                # ss bank carries it; later region-first
                                # matmuls overwrite via pending-zero.
                                nc.tensor.matmul(
                                    ss_ps[:, k, :B],
                                    sqt[:, j, 128 * k : 128 * (k + 1)],
                                    slt[:, j],
                                    start=(j == 0 and k == 0),
                                    stop=(j == TNT - 1 and k == KC - 1),
                                    skip_group_check=True,
                                )
                    else:
                        nc.scalar.activation(
                            sqv[:, j0:j1].rearrange("p j c -> p (j c)"),
                            vn[:, j0:j1].rearrange("p j c -> p (j c)"),
                            SQ,
                        )
                        for j in range(j0, j1):
                            for k in range(KC):
                                nc.tensor.matmul(
                                    ss_ps[:, k, B:],
                                    sqv[:, j, 128 * k : 128 * (k + 1)],
                                    slv[:, j],
                                    start=False,
                                    stop=(j == VNT - 1 and k == KC - 1),
                                    skip_group_check=True,
                                )

                # ---- rnorm factors + scaled bf16 operands ----
                rnt = norm_pool.tile([128, KC, B], f32, tag="rnt")
                rnv = norm_pool.tile([128, KC, YS], f32, tag="rnv")
                rnt_x = ops_pool.tile([128, KC, B, T], bf16, tag="rnt_x")
                rnv_x = ops_pool.tile([128, KC, YS, IPAD], bf16, tag="rnv_x")
                tl = ops_pool.tile([128, KC, B, T], bf16, tag="tl")
                ve = ops_pool.tile([128, KC, YS, IPAD], bf16, tag="ve")

                for k in range(KC):
                    # text: rnt = 1/sqrt(ss)
                    nc.scalar.activation(rnt[:, k], ss_ps[:, k, :B], SQRT)
                    nc.vector.reciprocal(rnt[:, k], rnt[:, k])
                    nc.scalar.activation(
                        rnt_x[:, k],
                        rnt[:, k].unsqueeze(2).broadcast_to((128, B, T)),
                        CP,
                    )
                    nc.vector.tensor_mul(tl[:, k], ttl[:, k], rnt_x[:, k])
                    # video: rnv = temp/sqrt(ss) = 1/sqrt(ss/temp^2)
                    nc.scalar.activation(
                        rnv[:, k],
                        ss_ps[:, k, B:],
                        SQRT,
                        scale=1.0 / (TEMPERATURE**2),
                    )
                    nc.vector.reciprocal(rnv[:, k], rnv[:, k])
                    nc.scalar.activation(
                        rnv_x[:, k],
                        rnv[:, k].unsqueeze(2).broadcast_to((128, YS, IPAD)),
                        CP,
                    )
                    nc.vector.tensor_mul(ve[:, k], vtt[:, k], rnv_x[:, k])
                    # keep the PE array warm across the norm->scores gap
                    nc.tensor.matmul(
                        wup_ps[:, :512],
                        ttl[:, k, 0:4, :].rearrange("p b t -> p (b t)"),
                        rnt_x[:, k].rearrange("p b t -> p (b t)")[:, :512],
                        start=True,
                        stop=True,
                        skip_group_check=True,
                    )

            # ---- scores + max over image tokens + masked mean ----
            tlf = tl.rearrange("p k b t -> p k (b t)")
            with tc.tile_pool(name="simps", bufs=3, space="PSUM") as simps_pool:
                for m in range(MT):
                    ps = [
                        simps_pool.tile(
                            [128, 2, 512], f32, tag="ps", name=f"ps{m}_{h}"
                        )
                        for h in range(2)
                    ]
                    for k in range(KC):
                        lhsT = tlf[:, k, m * 128 : (m + 1) * 128]
                        for j in range(4):  # 2 videos per psum bank
                            nc.tensor.matmul(
                                ps[j // 2][:, j % 2, : 2 * IPAD],
                                lhsT,
                                ve[:, k, 2 * j : 2 * j + 2].rearrange(
                                    "p y i -> p (y i)"
                                ),
                                start=(k == 0),
                                stop=(k == KC - 1),
                                skip_group_check=True,
                            )
                    t2i_m = t2i_pool.tile([128, YS], f32, tag="t2i", name=f"t2i{m}")
                    for h in range(2):
                        nc.vector.reduce_max(
                            out=t2i_m[:, 4 * h : 4 * h + 4].rearrange(
                                "p (a y) -> p a y", a=2
                            ),
                            in_=ps[h][:, :, : 2 * IPAD]
                            .rearrange("p a (y i) -> p a y i", y=2)[:, :, :, :I1],
                            axis=X,
                        )
                    nc.tensor.matmul(
                        loss_ps[:, :],
                        wt[:, m],
                        t2i_m[:],
                        start=(m == 0),
                        stop=(m == MT - 1),
                        skip_group_check=True,
                    )

                osb = osb_pool.tile([B, YS], f32, tag="osb")
                nc.scalar.activation(osb[:], loss_ps[:], CP)
                nc.sync.dma_start(out=out, in_=osb[:])

    _split_multi_waits(nc)
    return nc


def _get_nc():
    if "nc" not in _CACHE:
        _CACHE["nc"] = build_nc()
    return _CACHE["nc"]


def host_prep(text_embeds, video_embeds, text_attn_mask):
    """Layout-only host prep: transposes, bf16 cast, padding, selectors, W."""
    bf16 = ml_dtypes.bfloat16

    # channel-major matmul operands
    tt = np.ascontiguousarray(text_embeds.transpose(2, 0, 1))  # [C, B, T1]
    tt_lat = np.ascontiguousarray(tt[:, :, 1:]).astype(bf16)
    vtr = video_embeds.transpose(2, 0, 1)  # [C, B, I1]
    vt_pad = np.zeros((C, B, IPAD), np.float32)
    vt_pad[:, :, :I1] = vtr
    vt_pad = vt_pad.astype(bf16)

    # natural-layout (token-major) copies for the norm selector matmuls
    tnat = np.zeros((TNT * 128, C), np.float32)
    tnat[:TNR] = text_embeds.reshape(TNR, C)
    tnat = tnat.astype(bf16)
    sel_t = np.zeros((TNT * 128, B), np.float32)
    rows = np.arange(TNR)
    sel_t[rows, rows // T1] = 1.0
    sel_t = sel_t.astype(bf16)

    sel_v = np.zeros((VNT * 128, YS), np.float32)
    vrows = np.arange(VNR)
    sel_v[vrows, vrows // I1] = 1.0
    sel_v = sel_v.astype(bf16)

    # masked-mean weight matrix
    mask = text_attn_mask[:, 1:].astype(np.float32)  # [B, T]
    cnt = np.maximum(mask.sum(axis=1), MEAN_EPS).astype(np.float32)
    wsel = np.zeros((M, B), np.float32)
    for x in range(B):
        wsel[x * T : (x + 1) * T, x] = mask[x] / cnt[x]

    in_maps = []
    for i in range(NCORES):
        vshard = video_embeds[i * YS : (i + 1) * YS]  # [YS, I1, C]
        vnat = np.zeros((VNT * 128, C), np.float32)
        vnat[:VNR] = vshard.reshape(VNR, C)
        in_maps.append(
            {
                "tt_lat": tt_lat,
                "vt": np.ascontiguousarray(vt_pad[:, i * YS : (i + 1) * YS, :]),
                "tnat": tnat,
                "vnat": vnat.astype(bf16),
                "sel_t": sel_t,
                "sel_v": sel_v,
                "wsel": wsel,
            }
        )
    return in_maps


def host_finish(t2i_slabs):
    """exp / diag / sum / log / mean on the [64, 64] text_to_image matrix."""
    t2i = np.concatenate(t2i_slabs, axis=1).astype(np.float32)  # [B, B]
    e = np.exp(t2i)
    pos = np.diagonal(e)
    den = e.sum(axis=-1)
    loss = -np.log(pos / den + LOG_EPS).mean()
    return np.array([loss], dtype=np.float32)


def kernel(text_embeds, video_embeds, text_attn_mask):
    from concourse import bass_utils

    nc = _get_nc()
    in_maps = host_prep(
        np.asarray(text_embeds, np.float32),
        np.asarray(video_embeds, np.float32),
        np.asarray(text_attn_mask),
    )
    res = bass_utils.run_bass_kernel_spmd(
        nc, in_maps, core_ids=list(range(NCORES))
    )
    return host_finish([res.results[i]["out"] for i in range(NCORES)])
